# revision 22
# baseline (speedup 1.0000x reference)
"""ConceptCLIP loss kernel for 8x Trainium2 NeuronCores (Bass/Tile).

Strategy (data-parallel over the image batch axis m):
  - Each core owns 16 of the 128 images; concept/text features (small) are
    replicated to every core. Host gathers/sums the per-element losses.
  - Concepts are host-packed: only the w < counts[v] concepts take part,
    cutting ~half the FLOPs. Patches and concepts are L2-normalized, scaled
    by 16 and quantized to fp8 e4m3 (TRN variant, max +-240) on the host,
    already laid out in the transposed (d-major) SBUF format the PE wants.
  - Device pipeline: big fp8 matmul A[concept, image-pair cols] with
    perf_mode=DoubleRow (2 fp8 weights per PE cell, K=256 per instruction;
    6 K-chunks -> 3 DR steps). 4 concurrent accumulation chains in 4 PSUM
    banks (2 images of 196 patch-columns per bank) so each weight load
    feeds 4 matmuls. DVE reduce_max over patches per image -> fp32 matmul
    with the host-built gather matrix G (mask / (256*counts)) -> logits ->
    softplus loss elements, summed on host. IT-align runs in bf16 from
    host-normalized pre-transposed CLS features.
"""

import math
import os
import sys

for _p in ("/opt/trn_rl_repo", "/root/.axon_site/_ro/trn_rl_repo"):
    if os.path.isdir(_p) and _p not in sys.path:
        sys.path.insert(0, _p)

import ml_dtypes
import numpy as np

import concourse.tile as tile
from concourse import bacc, mybir
from concourse.bass_utils import run_bass_kernel_spmd

BF16 = ml_dtypes.bfloat16
FP8 = ml_dtypes.float8_e4m3  # TRN FP8_EXP4-compatible (max +-240, has inf)

N_CORES = 8
B, NPATCH, D, W = 128, 196, 768, 32
M_PER = B // N_CORES   # 16 images per core
PAIRS = M_PER // 2     # 8 image pairs, one per PSUM-bank chain slot
KC = D // 128          # 6 contraction chunks of 128
KD = KC // 2           # 3 DoubleRow steps of K=256
NPAD = 400             # pair tile free width; k-substride 400B % 16 == 0

F32 = mybir.dt.float32
BF = mybir.dt.bfloat16
F8 = mybir.dt.float8e4
AX = mybir.AxisListType
AF = mybir.ActivationFunctionType
DR = mybir.MatmulPerfMode.DoubleRow

_cache = {}


def _build(C):
    """Build + compile the per-core Bass program. C = number of 128-row packed
    concept chunks. The logits affine + softplus run on the host."""
    nc = bacc.Bacc("TRN2", target_bir_lowering=False, debug=False,
                   num_devices=N_CORES)

    d_rhs = nc.dram_tensor("rhs", (PAIRS, 128, KC, NPAD), F8, kind="ExternalInput")
    d_cTd = nc.dram_tensor("cTd", (C, 128, KC, 128), F8, kind="ExternalInput")
    d_GT = nc.dram_tensor("GT", (128, C, B), BF, kind="ExternalInput")
    d_ti = nc.dram_tensor("tiT", (128, KC, B + M_PER), BF, kind="ExternalInput")
    d_S = nc.dram_tensor("s_out", (128, M_PER), F32, kind="ExternalOutput")
    d_IT = nc.dram_tensor("it_out", (128, M_PER), F32, kind="ExternalOutput")

    with tile.TileContext(nc) as tc:
        with (
            tc.tile_pool(name="consts", bufs=1) as consts,
            tc.tile_pool(name="small", bufs=4) as small,
            tc.tile_pool(name="psum", bufs=2, space="PSUM") as psum,
        ):
            # DMA issues cost ~0.65us apiece on an engine's DGE, so spread
            # them over gpsimd/scalar/sync, ordered by when the PE needs
            # the data: cTd[0] + txt/img first, the 8 rhs pairs, then the
            # remaining concept chunks.
            cTd = consts.tile([128, C, KC, 128], F8, tag="cTd")
            rhs = [consts.tile([128, KC, NPAD], F8, tag=f"rhs{p}", name=f"rhs{p}")
                   for p in range(PAIRS)]
            ti = consts.tile([128, KC, B + M_PER], BF, tag="ti")
            GT = consts.tile([128, C, B], BF, tag="GT")
            scratch = consts.tile([128, 2, NPAD], F8, tag="scratch")
            maxcol = consts.tile([128, C, M_PER], BF, tag="maxcol")

            # one engine = one hardware DMA queue at full HBM rate; parallel
            # queues just split the bandwidth. So: a single sync-issued queue,
            # strictly ordered by when the PE consumes each tensor.
            # startup is DMA-issue-rate bound (~0.65us per HWDGE issue), so
            # interleave the critical head of the schedule across the sync
            # and scalar DGEs; both queues share HBM but transfers are short
            nc.gpsimd.memset(scratch[:], 0.0)
            nc.gpsimd.dma_start(out=ti[:], in_=d_ti.ap())
            nc.sync.dma_start(out=cTd[:, 0], in_=d_cTd.ap()[0])
            # pairs 0-3: ship the first DoubleRow step's k-pair first so the
            # opening chain starts ~1.5us earlier, then the remainder
            for p in range(4):
                eng = nc.sync if p % 2 == 0 else nc.scalar
                eng.dma_start(out=rhs[p][:, 0:2, :],
                              in_=d_rhs.ap()[p][:, 0:2, :])
            for p in range(4):
                eng = nc.sync if p % 2 == 0 else nc.scalar
                eng.dma_start(out=rhs[p][:, 2:KC, :],
                              in_=d_rhs.ap()[p][:, 2:KC, :])
            nc.sync.dma_start(out=cTd[:, 1], in_=d_cTd.ap()[1])
            nc.scalar.dma_start(out=cTd[:, 2], in_=d_cTd.ap()[2])
            for p in range(4, PAIRS):
                eng = nc.sync if p % 2 == 0 else nc.scalar
                eng.dma_start(out=cTd[:, p - 1], in_=d_cTd.ap()[p - 1])
                eng.dma_start(out=rhs[p][:], in_=d_rhs.ap()[p])
            for c in range(PAIRS - 1, C):
                eng = nc.sync if c % 2 == 0 else nc.scalar
                eng.dma_start(out=cTd[:, c], in_=d_cTd.ap()[c])
            nc.sync.dma_start(out=GT[:], in_=d_GT.ap())

            # dummy matmuls on zeroed scratch ramp the PE p-state out of the
            # low-clock regime while the first DMAs are still in flight
            wps = psum.tile([128, 4, 512], F32, tag="ps", name="ps4")

            def warm_mm(n):
                for i in range(n):
                    nc.tensor.matmul(wps[:, 1 + i % 3, 0:2 * NPATCH],
                                     lhsT=scratch[:, :, 0:128],
                                     rhs=scratch[:, :, 0:2 * NPATCH],
                                     start=True, stop=True, perf_mode=DR)

            warm_mm(6)

            # IT-align logits into the warm tile's bank 0 (no new PSUM alloc,
            # so nothing downstream waits on it); shipped via the idle queues
            for k in range(KC):
                nc.tensor.matmul(wps[:, 0, 0:M_PER], lhsT=ti[:, k, 0:B],
                                 rhs=ti[:, k, B:B + M_PER], start=(k == 0),
                                 stop=(k == KC - 1))
            it_sb = small.tile([128, M_PER], F32, tag="it_sb")
            nc.vector.tensor_copy(out=it_sb[:], in_=wps[:, 0, 0:M_PER])
            nc.scalar.dma_start(out=d_IT.ap(), in_=it_sb[:])

            def main_pt(pt, warm_after_j=0):
                # A[concept, pair cols] with 4 chains in 4 PSUM banks; each
                # DoubleRow weight load (K=256) feeds 4 matmuls of 392 cols.
                for c in range(C):
                    ps4 = psum.tile([128, 4, 512], F32, tag="ps", name="ps4")
                    for j in range(KD):
                        for i in range(4):
                            nc.tensor.matmul(
                                ps4[:, i, 0:2 * NPATCH],
                                lhsT=cTd[:, c, 2 * j:2 * j + 2, :],
                                rhs=rhs[pt * 4 + i][:, 2 * j:2 * j + 2, 0:2 * NPATCH],
                                start=(j == 0), stop=(j == KD - 1),
                                perf_mode=DR)
                        # keep the PE pipe hot across the first chain's
                        # split-DMA wait (idle resets the p-state clock)
                        if c == 0 and warm_after_j and j == 0:
                            warm_mm(warm_after_j)
                    nc.vector.reduce_max(
                        out=maxcol[:, c, pt * 8:pt * 8 + 8],
                        in_=ps4[:, :, 0:2 * NPATCH].rearrange(
                            "p b (s n) -> p b s n", s=2),
                        axis=AX.X)

            main_pt(0, warm_after_j=4)
            main_pt(1)

            # S[v, m] = sum_p G_eff[v, p] * maxcol[p, m]  (bf16)
            sps = psum.tile([128, 4, 512], F32, tag="ps", name="ps4")
            for c in range(C):
                nc.tensor.matmul(sps[:, 0, 0:M_PER], lhsT=GT[:, c, :],
                                 rhs=maxcol[:, c, :], start=(c == 0),
                                 stop=(c == C - 1))
            s_sb = small.tile([128, M_PER], F32, tag="s_sb")
            nc.vector.tensor_copy(out=s_sb[:], in_=sps[:, 0, 0:M_PER])
            nc.sync.dma_start(out=d_S.ap(), in_=s_sb[:])

    nc.compile()
    return nc


def _install_trace_hook():
    """Register the axon NTFF profiling hook (missing from this image) so
    run_bass_kernel_spmd(trace=True) can capture HW exec time."""
    import contextlib
    import ctypes
    import types

    import concourse.bass_utils as bu

    if "antenv.axon_hooks" in sys.modules:
        return
    so_path = "/opt/axon/libaxon_pjrt.so"

    def _make_hook():
        lib = ctypes.CDLL(so_path)
        if not hasattr(lib, "axon_start_nrt_profile"):
            return None
        lib.axon_start_nrt_profile.argtypes = [ctypes.POINTER(ctypes.c_int64),
                                               ctypes.c_size_t]
        lib.axon_start_nrt_profile.restype = ctypes.c_int64
        lib.axon_stop_nrt_profile.argtypes = [ctypes.c_char_p]
        lib.axon_stop_nrt_profile.restype = ctypes.c_int64

        @contextlib.contextmanager
        def _hook(output_dir, device_ids):
            import jax
            jax.devices()
            if device_ids:
                ids = (ctypes.c_int64 * len(device_ids))(*device_ids)
                rc = lib.axon_start_nrt_profile(ids, len(device_ids))
            else:
                rc = lib.axon_start_nrt_profile(None, 0)
            if rc != 0:
                raise RuntimeError(f"axon_start_nrt_profile rc={rc}")
            try:
                yield
            finally:
                n = lib.axon_stop_nrt_profile(str(output_dir).encode())
                print(f"profile: {n} file(s) written to {output_dir}",
                      file=sys.stderr)

        return _hook

    mod = types.ModuleType("antenv.axon_hooks")
    mod.get_axon_ntff_profile_hook = _make_hook
    sys.modules["antenv.axon_hooks"] = mod
    bu.upload_artifacts = lambda tmpdir: tmpdir  # no S3 in this container


def _l2n(x):
    n = np.sqrt((x * x).sum(-1, keepdims=True))
    return x / np.maximum(n, 1e-12)


def _prepare(inputs):
    image_features = np.asarray(inputs["image_features"], np.float32)
    text_features = np.asarray(inputs["text_features"], np.float32)
    image_token_features = np.asarray(inputs["image_token_features"], np.float32)
    concept_text_features = np.asarray(inputs["concept_text_features"], np.float32)
    counts = np.asarray(inputs["concept_counts"]).astype(np.int64)
    t = float(np.exp(np.clip(np.float32(inputs["logit_scale"]), -10.0, 10.0)))
    bias = float(np.float32(inputs["logit_bias"]))

    # pack concepts: keep only w < counts[v]; normalize, scale by 16, fp8
    vidx = np.repeat(np.arange(B), counts)
    widx = np.concatenate([np.arange(c) for c in counts])
    P = len(vidx)
    C = math.ceil(P / 128)
    Ppad = C * 128
    cnat = np.ones((Ppad, D), np.float32)
    cnat[:P] = concept_text_features[vidx, widx]
    c8 = (16.0 * _l2n(cnat)).astype(FP8)
    # cTd[c, p, k, m] = c8[c*128+m, k*128+p]
    cTd = np.ascontiguousarray(
        c8.reshape(C, 128, KC, 128).transpose(0, 3, 2, 1))

    # G_eff[v, p] = 1/(256*counts[v]) for packed concept p of sample v
    G = np.zeros((Ppad, B), np.float32)
    G[np.arange(P), vidx] = 1.0 / (256.0 * counts[vidx])
    # GT[p_lane, c, v] = G[c*128 + p_lane, v]
    GT = np.ascontiguousarray(
        G.reshape(C, 128, B).transpose(1, 0, 2)).astype(BF16)

    # patches: normalize rows, scale 16, fp8, transpose to (img, d, k, n),
    # pack image pairs side by side in a 400-wide tile (cols 392:400 unused)
    p8 = (16.0 * _l2n(image_token_features)).astype(FP8)
    p8 = p8.reshape(B, NPATCH, KC, 128).transpose(0, 3, 2, 1)  # (B,128,KC,N)
    rhs_all = np.zeros((B // 2, 128, KC, NPAD), FP8)
    rhs_all[:, :, :, 0:NPATCH] = p8[0::2]
    rhs_all[:, :, :, NPATCH:2 * NPATCH] = p8[1::2]

    # CLS features: normalized bf16, transposed; txt columns then the
    # core's img columns in one tensor/DMA
    txt = _l2n(text_features).astype(BF16)
    txtT = np.ascontiguousarray(txt.reshape(B, KC, 128).transpose(2, 1, 0))
    img = _l2n(image_features).astype(BF16)
    imgT_all = img.reshape(B, KC, 128).transpose(2, 1, 0)  # (128, KC, B)

    in_maps = []
    for core in range(N_CORES):
        s = slice(core * M_PER, (core + 1) * M_PER)
        in_maps.append({
            "rhs": np.ascontiguousarray(rhs_all[core * PAIRS:(core + 1) * PAIRS]),
            "cTd": cTd,
            "GT": GT,
            "tiT": np.ascontiguousarray(
                np.concatenate([txtT, imgT_all[:, :, s]], axis=2)),
        })
    return in_maps, C, t, bias


def _softplus_sum(logits_vm, core):
    """sum over (v, m) of softplus(-z * logits) with z=+1 on the diagonal
    (global image index core*M_PER+m == v), z=-1 elsewhere."""
    y = np.clip(logits_vm, -50.0, 50.0)
    el = np.logaddexp(0.0, y)  # z=-1 branch: softplus(+logit)
    idx = np.arange(M_PER)
    el[core * M_PER + idx, idx] = np.logaddexp(0.0, -y[core * M_PER + idx, idx])
    return float(el.sum())


def _run(inputs, trace=False, tmpdir=None):
    in_maps, C, t, bias = _prepare(inputs)
    if C not in _cache:
        _cache[C] = _build(C)
    nc = _cache[C]
    kwargs = {}
    if trace:
        _install_trace_hook()
        kwargs = dict(trace=True, tmpdir=tmpdir)
    res = run_bass_kernel_spmd(nc, in_maps, core_ids=list(range(N_CORES)),
                               **kwargs)
    it_sum = 0.0
    rc_sum = 0.0
    for core, r in enumerate(res.results):
        s_log = t * r["s_out"].astype(np.float64) + bias
        it_log = t * r["it_out"].astype(np.float64) + bias
        rc_sum += _softplus_sum(s_log, core)
        it_sum += _softplus_sum(it_log, core)
    it_loss = it_sum / (B * B)
    rc_loss = rc_sum / (B * B)
    total = it_loss + 0.5 * rc_loss
    out = (np.float32(total), np.float32(it_loss), np.float32(rc_loss))
    return out, res


def kernel(**inputs):
    out, _ = _run(inputs)
    return out


# revision 25
# speedup vs baseline: 1.0019x; 1.0019x over previous
"""ConceptCLIP loss kernel for 8x Trainium2 NeuronCores (Bass/Tile).

Strategy (data-parallel over the image batch axis m):
  - Each core owns 16 of the 128 images; concept/text features (small) are
    replicated to every core. Host gathers/sums the per-element losses.
  - Concepts are host-packed: only the w < counts[v] concepts take part,
    cutting ~half the FLOPs. Patches and concepts are L2-normalized, scaled
    by 16 and quantized to fp8 e4m3 (TRN variant, max +-240) on the host,
    already laid out in the transposed (d-major) SBUF format the PE wants.
  - Device pipeline: big fp8 matmul A[concept, image-pair cols] with
    perf_mode=DoubleRow (2 fp8 weights per PE cell, K=256 per instruction;
    6 K-chunks -> 3 DR steps). 4 concurrent accumulation chains in 4 PSUM
    banks (2 images of 196 patch-columns per bank) so each weight load
    feeds 4 matmuls. DVE reduce_max over patches per image -> fp32 matmul
    with the host-built gather matrix G (mask / (256*counts)) -> logits ->
    softplus loss elements, summed on host. IT-align runs in bf16 from
    host-normalized pre-transposed CLS features.
"""

import math
import os
import sys

for _p in ("/opt/trn_rl_repo", "/root/.axon_site/_ro/trn_rl_repo"):
    if os.path.isdir(_p) and _p not in sys.path:
        sys.path.insert(0, _p)

import ml_dtypes
import numpy as np

import concourse.tile as tile
from concourse import bacc, mybir
from concourse.bass_utils import run_bass_kernel_spmd

BF16 = ml_dtypes.bfloat16
FP8 = ml_dtypes.float8_e4m3  # TRN FP8_EXP4-compatible (max +-240, has inf)

N_CORES = 8
B, NPATCH, D, W = 128, 196, 768, 32
M_PER = B // N_CORES   # 16 images per core
PAIRS = M_PER // 2     # 8 image pairs, one per PSUM-bank chain slot
KC = D // 128          # 6 contraction chunks of 128
KD = KC // 2           # 3 DoubleRow steps of K=256
NPAD = 400             # pair tile free width; k-substride 400B % 16 == 0

F32 = mybir.dt.float32
BF = mybir.dt.bfloat16
F8 = mybir.dt.float8e4
AX = mybir.AxisListType
AF = mybir.ActivationFunctionType
DR = mybir.MatmulPerfMode.DoubleRow

_cache = {}


def _build(C):
    """Build + compile the per-core Bass program. C = number of 128-row packed
    concept chunks. The logits affine + softplus run on the host."""
    nc = bacc.Bacc("TRN2", target_bir_lowering=False, debug=False,
                   num_devices=N_CORES)

    d_rhs = nc.dram_tensor("rhs", (PAIRS, 128, KC, NPAD), F8, kind="ExternalInput")
    d_cTd = nc.dram_tensor("cTd", (C, 128, KC, 128), F8, kind="ExternalInput")
    d_GT = nc.dram_tensor("GT", (128, C, B), BF, kind="ExternalInput")
    d_ti = nc.dram_tensor("tiT", (128, KC, B + M_PER), BF, kind="ExternalInput")
    d_S = nc.dram_tensor("s_out", (128, M_PER), F32, kind="ExternalOutput")
    d_IT = nc.dram_tensor("it_out", (128, M_PER), F32, kind="ExternalOutput")

    with tile.TileContext(nc) as tc:
        with (
            tc.tile_pool(name="consts", bufs=1) as consts,
            tc.tile_pool(name="small", bufs=4) as small,
            tc.tile_pool(name="psum", bufs=2, space="PSUM") as psum,
        ):
            # DMA issues cost ~0.65us apiece on an engine's DGE, so spread
            # them over gpsimd/scalar/sync, ordered by when the PE needs
            # the data: cTd[0] + txt/img first, the 8 rhs pairs, then the
            # remaining concept chunks.
            cTd = consts.tile([128, C, KC, 128], F8, tag="cTd")
            rhs = [consts.tile([128, KC, NPAD], F8, tag=f"rhs{p}", name=f"rhs{p}")
                   for p in range(PAIRS)]
            ti = consts.tile([128, KC, B + M_PER], BF, tag="ti")
            GT = consts.tile([128, C, B], BF, tag="GT")
            scratch = consts.tile([128, 2, NPAD], F8, tag="scratch")
            maxcol = consts.tile([128, C, M_PER], BF, tag="maxcol")

            # one engine = one hardware DMA queue at full HBM rate; parallel
            # queues just split the bandwidth. So: a single sync-issued queue,
            # strictly ordered by when the PE consumes each tensor.
            # one engine = one hardware DMA queue at full HBM rate; splitting
            # across DGEs measures slower. Single sync queue, strictly in
            # consumption order.
            nc.gpsimd.memset(scratch[:], 0.0)
            nc.gpsimd.dma_start(out=ti[:], in_=d_ti.ap())
            nc.sync.dma_start(out=cTd[:, 0], in_=d_cTd.ap()[0])
            # pairs 0-3: ship the first DoubleRow step's k-pair first so the
            # opening chain starts ~1.5us earlier, then the remainder
            for p in range(4):
                nc.sync.dma_start(out=rhs[p][:, 0:2, :],
                                  in_=d_rhs.ap()[p][:, 0:2, :])
            for p in range(4):
                nc.sync.dma_start(out=rhs[p][:, 2:KC, :],
                                  in_=d_rhs.ap()[p][:, 2:KC, :])
            nc.sync.dma_start(out=cTd[:, 1], in_=d_cTd.ap()[1])
            nc.sync.dma_start(out=cTd[:, 2], in_=d_cTd.ap()[2])
            for p in range(4, PAIRS):
                nc.sync.dma_start(out=cTd[:, p - 1], in_=d_cTd.ap()[p - 1])
                nc.sync.dma_start(out=rhs[p][:], in_=d_rhs.ap()[p])
            for c in range(PAIRS - 1, C):
                nc.sync.dma_start(out=cTd[:, c], in_=d_cTd.ap()[c])
            nc.sync.dma_start(out=GT[:], in_=d_GT.ap())

            # dummy matmuls on zeroed scratch ramp the PE p-state out of the
            # low-clock regime while the first DMAs are still in flight
            wps = psum.tile([128, 4, 512], F32, tag="ps", name="ps4")

            def warm_mm(n):
                for i in range(n):
                    nc.tensor.matmul(wps[:, 1 + i % 3, 0:2 * NPATCH],
                                     lhsT=scratch[:, :, 0:128],
                                     rhs=scratch[:, :, 0:2 * NPATCH],
                                     start=True, stop=True, perf_mode=DR)

            warm_mm(6)

            # IT-align logits into the warm tile's bank 0 (no new PSUM alloc,
            # so nothing downstream waits on it); shipped via the idle queues
            for k in range(KC):
                nc.tensor.matmul(wps[:, 0, 0:M_PER], lhsT=ti[:, k, 0:B],
                                 rhs=ti[:, k, B:B + M_PER], start=(k == 0),
                                 stop=(k == KC - 1))
            it_sb = small.tile([128, M_PER], F32, tag="it_sb")
            nc.vector.tensor_copy(out=it_sb[:], in_=wps[:, 0, 0:M_PER])
            nc.scalar.dma_start(out=d_IT.ap(), in_=it_sb[:])

            def main_pt(pt, warm_after_j=0):
                # A[concept, pair cols] with 4 chains in 4 PSUM banks; each
                # DoubleRow weight load (K=256) feeds 4 matmuls of 392 cols.
                for c in range(C):
                    ps4 = psum.tile([128, 4, 512], F32, tag="ps", name="ps4")
                    for j in range(KD):
                        for i in range(4):
                            nc.tensor.matmul(
                                ps4[:, i, 0:2 * NPATCH],
                                lhsT=cTd[:, c, 2 * j:2 * j + 2, :],
                                rhs=rhs[pt * 4 + i][:, 2 * j:2 * j + 2, 0:2 * NPATCH],
                                start=(j == 0), stop=(j == KD - 1),
                                perf_mode=DR)
                        # keep the PE pipe hot across the first chain's
                        # split-DMA waits (idle resets the p-state clock)
                        if c == 0 and warm_after_j and j < KD - 1:
                            warm_mm(warm_after_j)
                    nc.vector.reduce_max(
                        out=maxcol[:, c, pt * 8:pt * 8 + 8],
                        in_=ps4[:, :, 0:2 * NPATCH].rearrange(
                            "p b (s n) -> p b s n", s=2),
                        axis=AX.X)

            main_pt(0, warm_after_j=3)
            main_pt(1)

            # S[v, m] = sum_p G_eff[v, p] * maxcol[p, m]  (bf16)
            sps = psum.tile([128, 4, 512], F32, tag="ps", name="ps4")
            for c in range(C):
                nc.tensor.matmul(sps[:, 0, 0:M_PER], lhsT=GT[:, c, :],
                                 rhs=maxcol[:, c, :], start=(c == 0),
                                 stop=(c == C - 1))
            s_sb = small.tile([128, M_PER], F32, tag="s_sb")
            nc.vector.tensor_copy(out=s_sb[:], in_=sps[:, 0, 0:M_PER])
            nc.sync.dma_start(out=d_S.ap(), in_=s_sb[:])

    nc.compile()
    return nc


def _install_trace_hook():
    """Register the axon NTFF profiling hook (missing from this image) so
    run_bass_kernel_spmd(trace=True) can capture HW exec time."""
    import contextlib
    import ctypes
    import types

    import concourse.bass_utils as bu

    if "antenv.axon_hooks" in sys.modules:
        return
    so_path = "/opt/axon/libaxon_pjrt.so"

    def _make_hook():
        lib = ctypes.CDLL(so_path)
        if not hasattr(lib, "axon_start_nrt_profile"):
            return None
        lib.axon_start_nrt_profile.argtypes = [ctypes.POINTER(ctypes.c_int64),
                                               ctypes.c_size_t]
        lib.axon_start_nrt_profile.restype = ctypes.c_int64
        lib.axon_stop_nrt_profile.argtypes = [ctypes.c_char_p]
        lib.axon_stop_nrt_profile.restype = ctypes.c_int64

        @contextlib.contextmanager
        def _hook(output_dir, device_ids):
            import jax
            jax.devices()
            if device_ids:
                ids = (ctypes.c_int64 * len(device_ids))(*device_ids)
                rc = lib.axon_start_nrt_profile(ids, len(device_ids))
            else:
                rc = lib.axon_start_nrt_profile(None, 0)
            if rc != 0:
                raise RuntimeError(f"axon_start_nrt_profile rc={rc}")
            try:
                yield
            finally:
                n = lib.axon_stop_nrt_profile(str(output_dir).encode())
                print(f"profile: {n} file(s) written to {output_dir}",
                      file=sys.stderr)

        return _hook

    mod = types.ModuleType("antenv.axon_hooks")
    mod.get_axon_ntff_profile_hook = _make_hook
    sys.modules["antenv.axon_hooks"] = mod
    bu.upload_artifacts = lambda tmpdir: tmpdir  # no S3 in this container


def _l2n(x):
    n = np.sqrt((x * x).sum(-1, keepdims=True))
    return x / np.maximum(n, 1e-12)


def _prepare(inputs):
    image_features = np.asarray(inputs["image_features"], np.float32)
    text_features = np.asarray(inputs["text_features"], np.float32)
    image_token_features = np.asarray(inputs["image_token_features"], np.float32)
    concept_text_features = np.asarray(inputs["concept_text_features"], np.float32)
    counts = np.asarray(inputs["concept_counts"]).astype(np.int64)
    t = float(np.exp(np.clip(np.float32(inputs["logit_scale"]), -10.0, 10.0)))
    bias = float(np.float32(inputs["logit_bias"]))

    # pack concepts: keep only w < counts[v]; normalize, scale by 16, fp8
    vidx = np.repeat(np.arange(B), counts)
    widx = np.concatenate([np.arange(c) for c in counts])
    P = len(vidx)
    C = math.ceil(P / 128)
    Ppad = C * 128
    cnat = np.ones((Ppad, D), np.float32)
    cnat[:P] = concept_text_features[vidx, widx]
    c8 = (16.0 * _l2n(cnat)).astype(FP8)
    # cTd[c, p, k, m] = c8[c*128+m, k*128+p]
    cTd = np.ascontiguousarray(
        c8.reshape(C, 128, KC, 128).transpose(0, 3, 2, 1))

    # G_eff[v, p] = 1/(256*counts[v]) for packed concept p of sample v
    G = np.zeros((Ppad, B), np.float32)
    G[np.arange(P), vidx] = 1.0 / (256.0 * counts[vidx])
    # GT[p_lane, c, v] = G[c*128 + p_lane, v]
    GT = np.ascontiguousarray(
        G.reshape(C, 128, B).transpose(1, 0, 2)).astype(BF16)

    # patches: normalize rows, scale 16, fp8, transpose to (img, d, k, n),
    # pack image pairs side by side in a 400-wide tile (cols 392:400 unused)
    p8 = (16.0 * _l2n(image_token_features)).astype(FP8)
    p8 = p8.reshape(B, NPATCH, KC, 128).transpose(0, 3, 2, 1)  # (B,128,KC,N)
    rhs_all = np.zeros((B // 2, 128, KC, NPAD), FP8)
    rhs_all[:, :, :, 0:NPATCH] = p8[0::2]
    rhs_all[:, :, :, NPATCH:2 * NPATCH] = p8[1::2]

    # CLS features: normalized bf16, transposed; txt columns then the
    # core's img columns in one tensor/DMA
    txt = _l2n(text_features).astype(BF16)
    txtT = np.ascontiguousarray(txt.reshape(B, KC, 128).transpose(2, 1, 0))
    img = _l2n(image_features).astype(BF16)
    imgT_all = img.reshape(B, KC, 128).transpose(2, 1, 0)  # (128, KC, B)

    in_maps = []
    for core in range(N_CORES):
        s = slice(core * M_PER, (core + 1) * M_PER)
        in_maps.append({
            "rhs": np.ascontiguousarray(rhs_all[core * PAIRS:(core + 1) * PAIRS]),
            "cTd": cTd,
            "GT": GT,
            "tiT": np.ascontiguousarray(
                np.concatenate([txtT, imgT_all[:, :, s]], axis=2)),
        })
    return in_maps, C, t, bias


def _softplus_sum(logits_vm, core):
    """sum over (v, m) of softplus(-z * logits) with z=+1 on the diagonal
    (global image index core*M_PER+m == v), z=-1 elsewhere."""
    y = np.clip(logits_vm, -50.0, 50.0)
    el = np.logaddexp(0.0, y)  # z=-1 branch: softplus(+logit)
    idx = np.arange(M_PER)
    el[core * M_PER + idx, idx] = np.logaddexp(0.0, -y[core * M_PER + idx, idx])
    return float(el.sum())


def _run(inputs, trace=False, tmpdir=None):
    in_maps, C, t, bias = _prepare(inputs)
    if C not in _cache:
        _cache[C] = _build(C)
    nc = _cache[C]
    kwargs = {}
    if trace:
        _install_trace_hook()
        kwargs = dict(trace=True, tmpdir=tmpdir)
    res = run_bass_kernel_spmd(nc, in_maps, core_ids=list(range(N_CORES)),
                               **kwargs)
    it_sum = 0.0
    rc_sum = 0.0
    for core, r in enumerate(res.results):
        s_log = t * r["s_out"].astype(np.float64) + bias
        it_log = t * r["it_out"].astype(np.float64) + bias
        rc_sum += _softplus_sum(s_log, core)
        it_sum += _softplus_sum(it_log, core)
    it_loss = it_sum / (B * B)
    rc_loss = rc_sum / (B * B)
    total = it_loss + 0.5 * rc_loss
    out = (np.float32(total), np.float32(it_loss), np.float32(rc_loss))
    return out, res


def kernel(**inputs):
    out, _ = _run(inputs)
    return out


# revision 26
# speedup vs baseline: 1.0300x; 1.0281x over previous
"""ConceptCLIP loss kernel for 8x Trainium2 NeuronCores (Bass/Tile).

Strategy (data-parallel over the image batch axis m):
  - Each core owns 16 of the 128 images; concept/text features (small) are
    replicated to every core. Host gathers/sums the per-element losses.
  - Concepts are host-packed: only the w < counts[v] concepts take part,
    cutting ~half the FLOPs. Patches and concepts are L2-normalized, scaled
    by 16 and quantized to fp8 e4m3 (TRN variant, max +-240) on the host,
    already laid out in the transposed (d-major) SBUF format the PE wants.
  - Device pipeline: big fp8 matmul A[concept, image-pair cols] with
    perf_mode=DoubleRow (2 fp8 weights per PE cell, K=256 per instruction;
    6 K-chunks -> 3 DR steps). 4 concurrent accumulation chains in 4 PSUM
    banks (2 images of 196 patch-columns per bank) so each weight load
    feeds 4 matmuls. DVE reduce_max over patches per image -> fp32 matmul
    with the host-built gather matrix G (mask / (256*counts)) -> logits ->
    softplus loss elements, summed on host. IT-align runs in bf16 from
    host-normalized pre-transposed CLS features.
"""

import math
import os
import sys

for _p in ("/opt/trn_rl_repo", "/root/.axon_site/_ro/trn_rl_repo"):
    if os.path.isdir(_p) and _p not in sys.path:
        sys.path.insert(0, _p)

import ml_dtypes
import numpy as np

import concourse.tile as tile
from concourse import bacc, mybir
from concourse.bass_utils import run_bass_kernel_spmd

BF16 = ml_dtypes.bfloat16
FP8 = ml_dtypes.float8_e4m3  # TRN FP8_EXP4-compatible (max +-240, has inf)

N_CORES = 8
B, NPATCH, D, W = 128, 196, 768, 32
M_PER = B // N_CORES   # 16 images per core
PAIRS = M_PER // 2     # 8 image pairs, one per PSUM-bank chain slot
KC = D // 128          # 6 contraction chunks of 128
KD = KC // 2           # 3 DoubleRow steps of K=256
NPAD = 400             # pair tile free width; k-substride 400B % 16 == 0

F32 = mybir.dt.float32
BF = mybir.dt.bfloat16
F8 = mybir.dt.float8e4
AX = mybir.AxisListType
AF = mybir.ActivationFunctionType
DR = mybir.MatmulPerfMode.DoubleRow

_cache = {}


def _build(C):
    """Build + compile the per-core Bass program. C = number of 128-row packed
    concept chunks. The logits affine + softplus run on the host."""
    nc = bacc.Bacc("TRN2", target_bir_lowering=False, debug=False,
                   num_devices=N_CORES)

    d_rhs = nc.dram_tensor("rhs", (PAIRS, 128, KC, NPAD), F8, kind="ExternalInput")
    d_cTd = nc.dram_tensor("cTd", (C, 128, KC, 128), F8, kind="ExternalInput")
    d_GT = nc.dram_tensor("GT", (128, C, B), BF, kind="ExternalInput")
    d_ti = nc.dram_tensor("tiT", (128, KC, B + M_PER), BF, kind="ExternalInput")
    d_S = nc.dram_tensor("s_out", (128, M_PER), F32, kind="ExternalOutput")
    d_IT = nc.dram_tensor("it_out", (128, M_PER), F32, kind="ExternalOutput")

    with tile.TileContext(nc) as tc:
        with (
            tc.tile_pool(name="consts", bufs=1) as consts,
            tc.tile_pool(name="small", bufs=4) as small,
            tc.tile_pool(name="stage", bufs=3) as stagep,
            tc.tile_pool(name="psum", bufs=2, space="PSUM") as psum,
        ):
            # DMA issues cost ~0.65us apiece on an engine's DGE, so spread
            # them over gpsimd/scalar/sync, ordered by when the PE needs
            # the data: cTd[0] + txt/img first, the 8 rhs pairs, then the
            # remaining concept chunks.
            cTd = consts.tile([128, C, KC, 128], F8, tag="cTd")
            rhs = [consts.tile([128, KC, NPAD], F8, tag=f"rhs{p}", name=f"rhs{p}")
                   for p in range(PAIRS)]
            ti = consts.tile([128, KC, B + M_PER], BF, tag="ti")
            GT = consts.tile([128, C, B], BF, tag="GT")
            scratch = consts.tile([128, 2, NPAD], F8, tag="scratch")
            maxcol = consts.tile([128, C, M_PER], BF, tag="maxcol")

            # one engine = one hardware DMA queue at full HBM rate; parallel
            # queues just split the bandwidth. So: a single sync-issued queue,
            # strictly ordered by when the PE consumes each tensor.
            # one engine = one hardware DMA queue at full HBM rate; splitting
            # across DGEs measures slower. Single sync queue, strictly in
            # consumption order.
            nc.gpsimd.memset(scratch[:], 0.0)
            nc.gpsimd.dma_start(out=ti[:], in_=d_ti.ap())
            nc.sync.dma_start(out=cTd[:, 0], in_=d_cTd.ap()[0])
            # pairs 0-3: ship the first DoubleRow step's k-pair first so the
            # opening chain starts ~1.5us earlier, then the remainder
            for p in range(4):
                nc.sync.dma_start(out=rhs[p][:, 0:2, :],
                                  in_=d_rhs.ap()[p][:, 0:2, :])
            for p in range(4):
                nc.sync.dma_start(out=rhs[p][:, 2:KC, :],
                                  in_=d_rhs.ap()[p][:, 2:KC, :])
            nc.sync.dma_start(out=cTd[:, 1], in_=d_cTd.ap()[1])
            nc.sync.dma_start(out=cTd[:, 2], in_=d_cTd.ap()[2])
            for p in range(4, PAIRS):
                nc.sync.dma_start(out=cTd[:, p - 1], in_=d_cTd.ap()[p - 1])
                nc.sync.dma_start(out=rhs[p][:], in_=d_rhs.ap()[p])
            for c in range(PAIRS - 1, C):
                nc.sync.dma_start(out=cTd[:, c], in_=d_cTd.ap()[c])
            nc.sync.dma_start(out=GT[:], in_=d_GT.ap())

            # dummy matmuls on zeroed scratch ramp the PE p-state out of the
            # low-clock regime while the first DMAs are still in flight
            wps = psum.tile([128, 4, 512], F32, tag="ps", name="ps4")

            def warm_mm(n):
                for i in range(n):
                    nc.tensor.matmul(wps[:, 1 + i % 3, 0:2 * NPATCH],
                                     lhsT=scratch[:, :, 0:128],
                                     rhs=scratch[:, :, 0:2 * NPATCH],
                                     start=True, stop=True, perf_mode=DR)

            warm_mm(6)
            awarm = small.tile([1, 2], F32, tag="awarm")
            nc.vector.memset(awarm[:], 0.0)
            nc.scalar.copy(out=awarm[:, 0:1], in_=awarm[:, 1:2])

            # IT-align logits into the warm tile's bank 0 (no new PSUM alloc,
            # so nothing downstream waits on it); shipped via the idle queues
            for k in range(KC):
                nc.tensor.matmul(wps[:, 0, 0:M_PER], lhsT=ti[:, k, 0:B],
                                 rhs=ti[:, k, B:B + M_PER], start=(k == 0),
                                 stop=(k == KC - 1))
            it_sb = small.tile([128, M_PER], F32, tag="it_sb")
            nc.vector.tensor_copy(out=it_sb[:], in_=wps[:, 0, 0:M_PER])
            nc.scalar.dma_start(out=d_IT.ap(), in_=it_sb[:])

            def main_pt(pt, warm_after_j=0):
                # A[concept, pair cols] with 4 chains in 4 PSUM banks; each
                # DoubleRow weight load (K=256) feeds 4 matmuls of 392 cols.
                for c in range(C):
                    ps4 = psum.tile([128, 4, 512], F32, tag="ps", name="ps4")
                    for j in range(KD):
                        for i in range(4):
                            nc.tensor.matmul(
                                ps4[:, i, 0:2 * NPATCH],
                                lhsT=cTd[:, c, 2 * j:2 * j + 2, :],
                                rhs=rhs[pt * 4 + i][:, 2 * j:2 * j + 2, 0:2 * NPATCH],
                                start=(j == 0), stop=(j == KD - 1),
                                perf_mode=DR)
                        # keep the PE pipe hot across the first chain's
                        # split-DMA wait (idle resets the p-state clock)
                        if c == 0 and warm_after_j and j == 0:
                            warm_mm(warm_after_j)
                    # ACT drains PSUM into SBUF (~0.8us) so the PSUM slot
                    # recycles without waiting the 1.8us DVE reduce; the
                    # reduce then runs from SBUF, decoupled from the PE.
                    stg = stagep.tile([128, 4, 2, NPATCH], BF, tag="stg")
                    nc.scalar.copy(out=stg[:],
                                   in_=ps4[:, :, 0:2 * NPATCH].rearrange(
                                       "p b (s n) -> p b s n", s=2))
                    nc.vector.reduce_max(
                        out=maxcol[:, c, pt * 8:pt * 8 + 8],
                        in_=stg[:], axis=AX.X)

            main_pt(0, warm_after_j=4)
            main_pt(1)

            # S[v, m] = sum_p G_eff[v, p] * maxcol[p, m]  (bf16)
            sps = psum.tile([128, 4, 512], F32, tag="ps", name="ps4")
            for c in range(C):
                nc.tensor.matmul(sps[:, 0, 0:M_PER], lhsT=GT[:, c, :],
                                 rhs=maxcol[:, c, :], start=(c == 0),
                                 stop=(c == C - 1))
            s_sb = small.tile([128, M_PER], F32, tag="s_sb")
            nc.vector.tensor_copy(out=s_sb[:], in_=sps[:, 0, 0:M_PER])
            nc.sync.dma_start(out=d_S.ap(), in_=s_sb[:])

    nc.compile()
    return nc


def _install_trace_hook():
    """Register the axon NTFF profiling hook (missing from this image) so
    run_bass_kernel_spmd(trace=True) can capture HW exec time."""
    import contextlib
    import ctypes
    import types

    import concourse.bass_utils as bu

    if "antenv.axon_hooks" in sys.modules:
        return
    so_path = "/opt/axon/libaxon_pjrt.so"

    def _make_hook():
        lib = ctypes.CDLL(so_path)
        if not hasattr(lib, "axon_start_nrt_profile"):
            return None
        lib.axon_start_nrt_profile.argtypes = [ctypes.POINTER(ctypes.c_int64),
                                               ctypes.c_size_t]
        lib.axon_start_nrt_profile.restype = ctypes.c_int64
        lib.axon_stop_nrt_profile.argtypes = [ctypes.c_char_p]
        lib.axon_stop_nrt_profile.restype = ctypes.c_int64

        @contextlib.contextmanager
        def _hook(output_dir, device_ids):
            import jax
            jax.devices()
            if device_ids:
                ids = (ctypes.c_int64 * len(device_ids))(*device_ids)
                rc = lib.axon_start_nrt_profile(ids, len(device_ids))
            else:
                rc = lib.axon_start_nrt_profile(None, 0)
            if rc != 0:
                raise RuntimeError(f"axon_start_nrt_profile rc={rc}")
            try:
                yield
            finally:
                n = lib.axon_stop_nrt_profile(str(output_dir).encode())
                print(f"profile: {n} file(s) written to {output_dir}",
                      file=sys.stderr)

        return _hook

    mod = types.ModuleType("antenv.axon_hooks")
    mod.get_axon_ntff_profile_hook = _make_hook
    sys.modules["antenv.axon_hooks"] = mod
    bu.upload_artifacts = lambda tmpdir: tmpdir  # no S3 in this container


def _l2n(x):
    n = np.sqrt((x * x).sum(-1, keepdims=True))
    return x / np.maximum(n, 1e-12)


def _prepare(inputs):
    image_features = np.asarray(inputs["image_features"], np.float32)
    text_features = np.asarray(inputs["text_features"], np.float32)
    image_token_features = np.asarray(inputs["image_token_features"], np.float32)
    concept_text_features = np.asarray(inputs["concept_text_features"], np.float32)
    counts = np.asarray(inputs["concept_counts"]).astype(np.int64)
    t = float(np.exp(np.clip(np.float32(inputs["logit_scale"]), -10.0, 10.0)))
    bias = float(np.float32(inputs["logit_bias"]))

    # pack concepts: keep only w < counts[v]; normalize, scale by 16, fp8
    vidx = np.repeat(np.arange(B), counts)
    widx = np.concatenate([np.arange(c) for c in counts])
    P = len(vidx)
    C = math.ceil(P / 128)
    Ppad = C * 128
    cnat = np.ones((Ppad, D), np.float32)
    cnat[:P] = concept_text_features[vidx, widx]
    c8 = (16.0 * _l2n(cnat)).astype(FP8)
    # cTd[c, p, k, m] = c8[c*128+m, k*128+p]
    cTd = np.ascontiguousarray(
        c8.reshape(C, 128, KC, 128).transpose(0, 3, 2, 1))

    # G_eff[v, p] = 1/(256*counts[v]) for packed concept p of sample v
    G = np.zeros((Ppad, B), np.float32)
    G[np.arange(P), vidx] = 1.0 / (256.0 * counts[vidx])
    # GT[p_lane, c, v] = G[c*128 + p_lane, v]
    GT = np.ascontiguousarray(
        G.reshape(C, 128, B).transpose(1, 0, 2)).astype(BF16)

    # patches: normalize rows, scale 16, fp8, transpose to (img, d, k, n),
    # pack image pairs side by side in a 400-wide tile (cols 392:400 unused)
    p8 = (16.0 * _l2n(image_token_features)).astype(FP8)
    p8 = p8.reshape(B, NPATCH, KC, 128).transpose(0, 3, 2, 1)  # (B,128,KC,N)
    rhs_all = np.zeros((B // 2, 128, KC, NPAD), FP8)
    rhs_all[:, :, :, 0:NPATCH] = p8[0::2]
    rhs_all[:, :, :, NPATCH:2 * NPATCH] = p8[1::2]

    # CLS features: normalized bf16, transposed; txt columns then the
    # core's img columns in one tensor/DMA
    txt = _l2n(text_features).astype(BF16)
    txtT = np.ascontiguousarray(txt.reshape(B, KC, 128).transpose(2, 1, 0))
    img = _l2n(image_features).astype(BF16)
    imgT_all = img.reshape(B, KC, 128).transpose(2, 1, 0)  # (128, KC, B)

    in_maps = []
    for core in range(N_CORES):
        s = slice(core * M_PER, (core + 1) * M_PER)
        in_maps.append({
            "rhs": np.ascontiguousarray(rhs_all[core * PAIRS:(core + 1) * PAIRS]),
            "cTd": cTd,
            "GT": GT,
            "tiT": np.ascontiguousarray(
                np.concatenate([txtT, imgT_all[:, :, s]], axis=2)),
        })
    return in_maps, C, t, bias


def _softplus_sum(logits_vm, core):
    """sum over (v, m) of softplus(-z * logits) with z=+1 on the diagonal
    (global image index core*M_PER+m == v), z=-1 elsewhere."""
    y = np.clip(logits_vm, -50.0, 50.0)
    el = np.logaddexp(0.0, y)  # z=-1 branch: softplus(+logit)
    idx = np.arange(M_PER)
    el[core * M_PER + idx, idx] = np.logaddexp(0.0, -y[core * M_PER + idx, idx])
    return float(el.sum())


def _run(inputs, trace=False, tmpdir=None):
    in_maps, C, t, bias = _prepare(inputs)
    if C not in _cache:
        _cache[C] = _build(C)
    nc = _cache[C]
    kwargs = {}
    if trace:
        _install_trace_hook()
        kwargs = dict(trace=True, tmpdir=tmpdir)
    res = run_bass_kernel_spmd(nc, in_maps, core_ids=list(range(N_CORES)),
                               **kwargs)
    it_sum = 0.0
    rc_sum = 0.0
    for core, r in enumerate(res.results):
        s_log = t * r["s_out"].astype(np.float64) + bias
        it_log = t * r["it_out"].astype(np.float64) + bias
        rc_sum += _softplus_sum(s_log, core)
        it_sum += _softplus_sum(it_log, core)
    it_loss = it_sum / (B * B)
    rc_loss = rc_sum / (B * B)
    total = it_loss + 0.5 * rc_loss
    out = (np.float32(total), np.float32(it_loss), np.float32(rc_loss))
    return out, res


def kernel(**inputs):
    out, _ = _run(inputs)
    return out
